# revision 1
# baseline (speedup 1.0000x reference)
"""Bidirectional InfoNCE (CLIP-style) loss on 8 Trainium2 NeuronCores.

Data-parallel over the batch: core m owns rows [m*1024, (m+1)*1024) of the
similarity matrix and computes, for its row block:
  - rowlse_sum: sum_i log(sum_j exp(s_ij))          (scalar)
  - colsum:     sum_{i in block} exp(s_ij)          ([8192] partial)
  - diag_sum:   sum_i s_ii                          (scalar)
The host combines: loss = 0.5*(mean_row_lse + mean_col_lse) - mean_diag.

The program is identical on every core (true SPMD); all per-core variation
is carried by the input slices (a16 slice, td16 = matching t slice).
"""

import math
import os  # noqa: F401  (probe knobs)
from contextlib import ExitStack

import ml_dtypes
import numpy as np

import concourse.bass as bass
import concourse.tile as tile
from concourse import bacc, mybir
from concourse.bass import ts


class _Bacc(bacc.Bacc):
    """Bacc whose act-table pass is steered to the one set containing every
    activation function this kernel uses (Exp, Ln, Copy), so the loop never
    switches tables. Ids are positional, so competing sets are blanked
    rather than removed."""

    _ACT_SET = "natural_log_exp_and_others"

    def insert_act_table_loads(self):
        import bass_rust as _bass_rust
        from concourse.hw_specs import get_activation_tables

        has_activation = any(
            isinstance(i, mybir.InstActivation)
            for b in self.main_func.blocks
            for i in b.instructions
        )
        if not has_activation:
            return
        tables = []
        for name, funcs in get_activation_tables(self.m.arch).items():
            keep = name == self._ACT_SET
            tables.append((name, funcs if keep else set()))
        _bass_rust.insert_act_table_loads(self, tables)

B = 8192          # global batch
D = 1024          # embedding dim
NCORES = 8
BL = B // NCORES  # rows per core (1024)
TEMP = 0.07

P = 128           # partitions
KT = D // P       # 8 k-tiles over the contraction dim
IT = BL // P      # 8 i-tiles (local rows)
NJ = 512          # j-chunk width (matmul moving free dim)
JCH = B // NJ     # 16 j-chunks
JT = 4            # 128-row t-tiles per chunk

F32 = mybir.dt.float32
BF16 = mybir.dt.bfloat16
AF = mybir.ActivationFunctionType
OP = mybir.AluOpType

LN_INV_TEMP = math.log(1.0 / TEMP)


def _emit(tc: tile.TileContext, a16, t16, td16, colsum_out, scal_out,
          repeat=1):
    nc = tc.nc
    ctx = ExitStack()
    with ctx:
        singles = ctx.enter_context(tc.tile_pool(name="singles", bufs=1))
        dram = ctx.enter_context(tc.tile_pool(name="dram", bufs=1, space="DRAM"))

        ones16 = singles.tile([P, 1], BF16)
        nc.vector.memset(ones16, 1.0)
        ones32 = singles.tile([P, 1], F32)
        nc.vector.memset(ones32, 1.0)
        bias_lnT = singles.tile([P, 1], F32)
        nc.vector.memset(bias_lnT, LN_INV_TEMP)

        aT = singles.tile([P, KT, BL], BF16)      # a16 transposed: [d, k, i]
        scaleA = singles.tile([P, IT], F32)       # rA/T per local row
        diagv = singles.tile([P, IT], F32)        # diagonal logits
        rs = singles.tile([P, IT, JCH], F32)      # per-(row, chunk) exp sums
        colsum_sb = singles.tile([1, B], F32)
        t16n_dram = dram.tile([B, D], BF16)       # normalized t, for xbar reload

        # ---------------- Phase A + pipelined Phase B ----------------
        with (
            tc.tile_pool(name="aload", bufs=1) as aload,
            tc.tile_pool(name="asc", bufs=3) as asc,
            tc.tile_pool(name="astat", bufs=1) as astat,
            tc.tile_pool(name="tload", bufs=2) as tload,
            tc.tile_pool(name="tnorm", bufs=2) as tnorm,
            tc.tile_pool(name="tsc", bufs=4) as tsc,
            tc.tile_pool(name="tstat", bufs=4) as tstat,
            tc.tile_pool(name="ttp", bufs=2) as ttp,
            tc.tile_pool(name="texp", bufs=4) as texp,
            tc.tile_pool(name="tndram", bufs=3, space="DRAM") as tndram,
            tc.tile_pool(name="psum_mm", bufs=4, space="PSUM") as psum_mm,
            tc.tile_pool(name="psum_cs", bufs=2, space="PSUM") as psum_cs,
        ):
            probe = os.environ.get("BIDI_PROBE", "")
            scale_eng = (nc.gpsimd if os.environ.get("BIDI_SCALE") == "gpsimd"
                         else nc.vector)

            def t_prep(jc, d2, half):
                """Load 4 t-tiles, normalize, write into half of the pair's
                DRAM bounce buffer (transposed-read later, one XBAR per pair)."""
                tt4 = tload.tile([P, JT, D], BF16, tag="tt4")
                nc.sync.dma_start(
                    tt4, t16[ts(jc, NJ), :].rearrange("(j p) d -> p j d", p=P))
                tss = tstat.tile([P, JT], F32, tag="tss")
                for j4 in range(JT):
                    tsq = tsc.tile([P, D], BF16, tag="tsq")
                    nc.vector.tensor_mul(tsq, tt4[:, j4, :], tt4[:, j4, :])
                    nc.vector.reduce_sum(tss[:, j4:j4 + 1], tsq,
                                         axis=mybir.AxisListType.X)
                tln = tstat.tile([P, JT], F32, tag="tln")
                nc.scalar.activation(tln, tss, AF.Ln)
                rT4 = tstat.tile([P, JT], F32, tag="rT4")
                nc.scalar.activation(rT4, tln, AF.Exp, scale=-0.5)
                ttn4 = tnorm.tile([P, JT, D], BF16, tag="ttn4")
                for j4 in range(JT):
                    scale_eng.tensor_scalar_mul(
                        out=ttn4[:, j4, :], in0=tt4[:, j4, :],
                        scalar1=rT4[:, j4:j4 + 1])
                nc.sync.dma_start(
                    d2[ts(half, NJ), :].rearrange("(j p) d -> p j d", p=P),
                    ttn4)

            def pair_prep(pp):
                d2 = tndram.tile([4 * NJ, D], BF16, tag="dbuf4")
                for h in range(4):
                    t_prep(4 * pp + h, d2, h)
                if probe == "notr":
                    return None
                tT2 = ttp.tile([P, KT, 4 * NJ], BF16, tag="tT4")
                nc.sync.dma_start_transpose(tT2, d2)
                return tT2

            # a_nat first: it gates the scaleA chain that every exp needs
            a_nat = aload.tile([P, IT, D], BF16)
            nc.sync.dma_start(
                a_nat, a16[:, :].rearrange("(t p) d -> p t d", p=P))
            # aT[d, k, i] = a16[i, k*128+d] via one XBAR load
            nc.sync.dma_start_transpose(aT, a16)

            # a-norm scales, one i-tile at a time so scaleA[:,0] is ready
            # before the first exp (diag path deferred to the end)
            asumsq = astat.tile([P, IT], F32)
            for ti in range(IT):
                sq = asc.tile([P, D], BF16, tag="sq")
                nc.scalar.activation(sq, a_nat[:, ti, :], AF.Square,
                                     accum_out=asumsq[:, ti:ti + 1])
                # scaleA = exp(-0.5*ln(asumsq) + ln(1/T)) = 1/(||a||*T)
                alog = asc.tile([P, 1], F32, tag="alog")
                nc.scalar.activation(alog, asumsq[:, ti:ti + 1], AF.Ln)
                nc.scalar.activation(scaleA[:, ti:ti + 1], alog, AF.Exp,
                                     scale=-0.5, bias=bias_lnT)

            def emit_diag():
                """Diagonal logits via per-row dot(a, td); scheduled mid-loop
                where DVE has slack."""
                td_nat = aload.tile([P, IT, D], BF16)
                nc.sync.dma_start(
                    td_nat, td16[:, :].rearrange("(t p) d -> p t d", p=P))
                tdsumsq = astat.tile([P, IT], F32)
                adot = astat.tile([P, IT], F32)
                rTd = astat.tile([P, IT], F32)
                for ti in range(IT):
                    sq2 = asc.tile([P, D], BF16, tag="sq")
                    nc.scalar.activation(sq2, td_nat[:, ti, :], AF.Square,
                                         accum_out=tdsumsq[:, ti:ti + 1])
                    sq3 = asc.tile([P, D], BF16, tag="sq")
                    nc.vector.tensor_mul(sq3, a_nat[:, ti, :],
                                         td_nat[:, ti, :])
                    nc.vector.reduce_sum(adot[:, ti:ti + 1], sq3,
                                         axis=mybir.AxisListType.X)
                tdlog = astat.tile([P, IT], F32)
                nc.scalar.activation(tdlog, tdsumsq, AF.Ln)
                nc.scalar.activation(rTd, tdlog, AF.Exp, scale=-0.5)
                # diag logits = adot * (rA/T) * rTd
                nc.vector.tensor_mul(diagv, adot, scaleA)
                nc.vector.tensor_mul(diagv, diagv, rTd)

            def mm_body(jc, tT2):
                if probe in ("preps", "notr"):
                    return
                ps_cs = psum_cs.tile([1, NJ], F32, tag="ps_cs")
                e16s = []
                for ti in range(IT):
                    ps = psum_mm.tile([P, NJ], F32, tag="ps_mm")
                    for k in range(KT):
                        nc.tensor.matmul(
                            ps, aT[:, k, ts(ti, P)],
                            tT2[:, k, ts(jc % 4, NJ)],
                            start=(k == 0), stop=(k == KT - 1))
                    if probe == "mm":
                        continue
                    e16 = texp.tile([P, NJ], BF16, tag="e16")
                    nc.scalar.activation(
                        e16, ps, AF.Exp, scale=scaleA[:, ti:ti + 1],
                        accum_out=rs[:, ti, jc:jc + 1])
                    e16s.append(e16)
                    if ti >= 2 and probe != "nocs":
                        nc.tensor.matmul(
                            ps_cs, ones16, e16s[ti - 2], start=(ti == 2),
                            stop=False, skip_group_check=True)
                if probe in ("mm", "nocs"):
                    return
                for ti in (IT - 2, IT - 1):
                    nc.tensor.matmul(
                        ps_cs, ones16, e16s[ti],
                        start=False, stop=(ti == IT - 1),
                        skip_group_check=True)
                nc.scalar.copy(colsum_sb[:, ts(jc, NJ)], ps_cs)

            def body():
                q = [pair_prep(0), pair_prep(1)]
                for jc in range(JCH):
                    pp, half = divmod(jc, 4)
                    if half == 0 and pp + 2 < JCH // 4:
                        q.append(pair_prep(pp + 2))
                    if jc == 8:
                        emit_diag()
                    mm_body(jc, q[0])
                    if half == 3:
                        q.pop(0)

            if repeat > 1:
                with tc.For_i(0, repeat, 1):
                    body()
            else:
                body()

            # ---------------- Phase C: final reductions ----------------
            if probe:
                return
            fincol = singles.tile([P, 2], F32)
            rsum = singles.tile([P, IT], F32)
            for ti in range(IT):
                nc.vector.tensor_reduce(
                    out=rsum[:, ti:ti + 1], in_=rs[:, ti, :],
                    axis=mybir.AxisListType.X, op=OP.add)
            lse8 = singles.tile([P, IT], F32)
            nc.scalar.activation(lse8, rsum, AF.Ln)
            nc.vector.tensor_reduce(
                out=fincol[:, 0:1], in_=lse8, axis=mybir.AxisListType.X, op=OP.add)
            nc.vector.tensor_reduce(
                out=fincol[:, 1:2], in_=diagv, axis=mybir.AxisListType.X, op=OP.add)
            psf = psum_cs.tile([1, 2], F32, tag="psf")
            nc.tensor.matmul(psf, ones32, fincol, start=True, stop=True,
                             skip_group_check=True)
            scal_sb = singles.tile([1, 2], F32)
            nc.scalar.copy(scal_sb, psf)

            nc.sync.dma_start(colsum_out, colsum_sb)
            nc.sync.dma_start(scal_out, scal_sb)


_NC_CACHE = {}


def _build(repeat=1):
    if repeat in _NC_CACHE:
        return _NC_CACHE[repeat]
    nc = _Bacc("TRN2", target_bir_lowering=False, debug=False,
               num_devices=NCORES)
    a16 = nc.dram_tensor("a16", [BL, D], BF16, kind="ExternalInput").ap()
    t16 = nc.dram_tensor("t16", [B, D], BF16, kind="ExternalInput").ap()
    td16 = nc.dram_tensor("td16", [BL, D], BF16, kind="ExternalInput").ap()
    colsum_out = nc.dram_tensor("colsum_out", [1, B], F32,
                                kind="ExternalOutput").ap()
    scal_out = nc.dram_tensor("scal_out", [1, 2], F32,
                              kind="ExternalOutput").ap()
    with tile.TileContext(nc) as tc:
        _emit(tc, a16, t16, td16, colsum_out, scal_out, repeat=repeat)
    nc.compile()
    _NC_CACHE[repeat] = nc
    return nc


def make_in_maps(audio_embeds: np.ndarray, text_embeds: np.ndarray):
    a16 = np.asarray(audio_embeds, dtype=np.float32).astype(ml_dtypes.bfloat16)
    t16 = np.asarray(text_embeds, dtype=np.float32).astype(ml_dtypes.bfloat16)
    in_maps = []
    for m in range(NCORES):
        sl = slice(m * BL, (m + 1) * BL)
        in_maps.append({"a16": a16[sl], "t16": t16, "td16": t16[sl]})
    return in_maps


def combine(results):
    colsum = np.zeros((B,), np.float64)
    rowlse_sum = 0.0
    diag_sum = 0.0
    for m in range(NCORES):
        colsum += results[m]["colsum_out"].reshape(-1).astype(np.float64)
        sc = results[m]["scal_out"].reshape(-1)
        rowlse_sum += float(sc[0])
        diag_sum += float(sc[1])
    col_lse_mean = float(np.log(colsum).mean())
    loss = 0.5 * (rowlse_sum / B + col_lse_mean) - diag_sum / B
    return np.float32(loss)


def kernel(audio_embeds: np.ndarray, text_embeds: np.ndarray) -> np.ndarray:
    from concourse.bass_utils import run_bass_kernel_spmd

    nc = _build()
    in_maps = make_in_maps(audio_embeds, text_embeds)
    res = run_bass_kernel_spmd(nc, in_maps, list(range(NCORES)))
    return combine(res.results)



# revision 4
# speedup vs baseline: 1.1529x; 1.1529x over previous
"""Bidirectional InfoNCE (CLIP-style) loss on 8 Trainium2 NeuronCores, v4.

2x4 grid sharding + fp8 DoubleRow matmuls.
  core m = (p, q), p = m // 4, q = m % 4
  - audio rows [p*4096, (p+1)*4096) (AR), text rows [q*2048, (q+1)*2048) (TC)
  - computes the [AR, TC] similarity block in fp8 DoubleRow (contraction
    256/instr, 2 MACs/cell/cycle); text normalized pre-matmul (small side),
    audio norms folded into the exp() scale (rA/T per psum partition row).
  - diag slice [m*1024, (m+1)*1024) via separate elementwise path.

Outputs (combined on host): rowsum_out [1, AR] = sum_j exp(s_ij) partials,
colsum_out [1, TC] = sum_i exp(s_ij) partials, scal_out [1,1] = sum_i s_ii.
loss = 0.5*(mean log sum_q rowsum + mean log sum_p colsum) - diag/B.

Schedule notes:
  - psum tiles span 2 banks ([P, 2, 512]); one exp instruction drains both
    (amortizes the ACT access latency); 2 mm-"rounds" of 2 j-chunks each.
  - k2-outer / bank-half-inner mm order so consecutive MMs share LDWEIGHTS.
  - flat DMA loads (partition-major) everywhere the row order is local;
    asumsq is computed in flat order and bounced through DRAM to the
    matmul partition order (scaleA layout is fixed by psum partitions).
  - a-sumsq on ACT for early chunks (ACT idle pre-mm), DVE for late ones;
    diag products on Pool with DVE reduces; colsum partial accumulation
    alternates DVE/Pool chains, folded on PE at round end.
"""

import math
import os
from contextlib import ExitStack

import ml_dtypes
import numpy as np

import concourse.bass as bass
import concourse.tile as tile
from concourse import bacc, mybir
from concourse.bass import ts


class _Bacc(bacc.Bacc):
    """Pin the act-table set containing Exp/Ln/Copy/Square so the loop never
    switches tables (ids positional; competing sets blanked, not removed)."""

    _ACT_SET = "natural_log_exp_and_others"

    def insert_act_table_loads(self):
        import bass_rust as _bass_rust
        from concourse.hw_specs import get_activation_tables

        has_activation = any(
            isinstance(i, mybir.InstActivation)
            for b in self.main_func.blocks
            for i in b.instructions
        )
        if not has_activation:
            return
        tables = []
        for name, funcs in get_activation_tables(self.m.arch).items():
            keep = name == self._ACT_SET
            tables.append((name, funcs if keep else set()))
        _bass_rust.insert_act_table_loads(self, tables)


B = 8192          # global batch
D = 1024          # embedding dim
NCORES = 8
PR, PC = 2, 4     # core grid: rows x cols
AR = B // PR      # audio rows per core (4096)
TC = B // PC      # text rows per core (2048)
DG = B // NCORES  # diag rows per core (1024)
TEMP = 0.07

P = 128
KT = D // P       # 8 k-tiles of 128 over contraction
K2 = KT // 2      # 4 DoubleRow k-tiles of 256
IT = AR // P      # 32 i-tiles
NJ = 512          # j-chunk width (one psum bank)
JCH = TC // NJ    # 4 j-chunks
R2 = 2            # mm rounds (2 j-chunks each)
ACH = 4           # a-prep chunks of 1024 rows

F32 = mybir.dt.float32
BF16 = mybir.dt.bfloat16
FP8 = mybir.dt.float8e4
AF = mybir.ActivationFunctionType
OP = mybir.AluOpType
DR = mybir.MatmulPerfMode.DoubleRow
AXX = mybir.AxisListType.X

LN_INV_TEMP = math.log(1.0 / TEMP)

A_SUMSQ_ACT_CHUNKS = (0, 1)   # a-chunks whose sumsq runs on ACT (early ones)


def _nopool():
    return os.environ.get("BIDI_NOPOOL") == "1"


def _cols_engine(ti):
    if _nopool():
        return "dve"
    return "dve" if ti % 2 == 0 else "pool"


def _emit(tc: tile.TileContext, a16, t16, ad16, td16,
          rowsum_out, colsum_out, scal_out, repeat=1):
    nc = tc.nc
    ctx = ExitStack()
    with ctx:
        singles = ctx.enter_context(tc.tile_pool(name="singles", bufs=1))
        dram = ctx.enter_context(tc.tile_pool(name="dram", bufs=1,
                                              space="DRAM"))

        ones16 = singles.tile([P, 1], BF16)
        nc.vector.memset(ones16, 1.0)
        ones32 = singles.tile([P, 1], F32)
        nc.vector.memset(ones32, 1.0)
        bias_lnT = singles.tile([P, 1], F32)
        nc.vector.memset(bias_lnT, LN_INV_TEMP)

        aT8 = [singles.tile([P, KT, 1024], FP8, name=f"aT8_{i}")
               for i in range(ACH)]
        t8 = [singles.tile([P, KT, NJ], FP8, name=f"t8_{i}")
              for i in range(JCH)]
        scaleA = singles.tile([P, IT], F32)       # rA/T per local audio row
        asq_sb = singles.tile([1, AR], F32)       # sumsq, row order (PE path)
        asumsq = singles.tile([P, IT], F32)       # sumsq in psum-row order
        asq_dram = dram.tile([1, AR], F32)
        rs = singles.tile([P, IT, R2], F32)       # rowsum partials
        cacc_d = singles.tile([P, TC], BF16)      # colsum partial (DVE)
        cacc_p = None
        if not _nopool():
            cacc_p = singles.tile([P, TC], BF16)  # colsum partial (Pool)
        colsum_sb = singles.tile([1, TC], F32)
        rsum = singles.tile([P, IT], F32)
        adss = singles.tile([P, DG // P], BF16)
        tdss = singles.tile([P, DG // P], BF16)
        adot = singles.tile([P, DG // P], BF16)
        diagv = singles.tile([P, DG // P], F32)
        scal_sb = singles.tile([1, 1], F32)

        with (
            tc.tile_pool(name="asc", bufs=1) as asc,
            tc.tile_pool(name="atr", bufs=2) as atr,
            tc.tile_pool(name="tload", bufs=2) as tload,
            tc.tile_pool(name="tsc", bufs=1) as tsc,
            tc.tile_pool(name="tstat", bufs=4) as tstat,
            tc.tile_pool(name="tnorm", bufs=1) as tnorm,
            tc.tile_pool(name="ttp", bufs=1) as ttp,
            tc.tile_pool(name="tndram", bufs=2, space="DRAM") as tndram,
            tc.tile_pool(name="dpool", bufs=1) as dpool,
            tc.tile_pool(name="dsc", bufs=1) as dsc,
            tc.tile_pool(name="texp", bufs=3) as texp,
            tc.tile_pool(name="psum_mm", bufs=3, space="PSUM") as psum_mm,
            tc.tile_pool(name="psum_cs", bufs=1, space="PSUM") as psum_cs,
        ):
            def a_prep(c):
                """Chunk c (1024 audio rows): XBAR -> fp8; sumsq from the
                transposed tiles (DVE square + accumulating ones-matmul on
                PE -> row-ordered sumsq), permuted to psum-row order via a
                DRAM bounce."""
                for h in range(2):
                    aTb = atr.tile([P, KT, NJ], BF16, tag="aTb")
                    nc.sync.dma_start_transpose(
                        aTb, a16[ts(2 * c + h, NJ), :])
                    nc.vector.tensor_copy(aT8[c][:, :, ts(h, NJ)], aTb)
                    sqb = asc.tile([P, KT, NJ], BF16, tag="sqb")
                    nc.vector.tensor_mul(sqb, aTb, aTb)
                    ps_sq = psum_cs.tile([1, NJ], F32, tag="ps_cs")
                    for k in range(KT):
                        nc.tensor.matmul(ps_sq, ones16, sqb[:, k, :],
                                         start=(k == 0), stop=(k == KT - 1),
                                         skip_group_check=True)
                    nc.scalar.copy(asq_sb[:, ts(2 * c + h, NJ)], ps_sq)
                # row order -> psum-partition order via DRAM bounce
                nc.sync.dma_start(asq_dram[:, ts(c, 1024)],
                                  asq_sb[:, ts(c, 1024)])
                nc.sync.dma_start(
                    asumsq[:, ts(c, 8)],
                    asq_dram[:, ts(c, 1024)].rearrange(
                        "a (t p) -> (a p) t", p=P))
                alog = tstat.tile([P, 8], F32, tag="alog")
                nc.scalar.activation(alog, asumsq[:, ts(c, 8)], AF.Ln)
                nc.scalar.activation(scaleA[:, ts(c, 8)], alog, AF.Exp,
                                     scale=-0.5, bias=bias_lnT)

            def t_half(r):
                """512 text rows (flat load): normalize -> bounce -> XBAR ->
                fp8 into t8[r]."""
                tt = tload.tile([P, 4, D], BF16, tag="tt")
                nc.sync.dma_start(
                    tt, t16[ts(r, NJ), :].rearrange("(p j) d -> p j d", j=4))
                tss = tstat.tile([P, 4], BF16, tag="tss")
                scr = tsc.tile([P, 4, D], BF16, tag="tscr4")
                nc.vector.tensor_mul(scr, tt, tt)
                with nc.allow_low_precision(reason="bf16 sumsq"):
                    nc.vector.tensor_reduce(out=tss, in_=scr, axis=AXX,
                                            op=OP.add)
                tln = tstat.tile([P, 4], F32, tag="tln")
                nc.scalar.activation(tln, tss, AF.Ln)
                rT4 = tstat.tile([P, 4], F32, tag="rT4")
                nc.scalar.activation(rT4, tln, AF.Exp, scale=-0.5)
                ttn = tnorm.tile([P, 4, D], BF16, tag="ttn")
                for s in range(4):
                    nc.vector.tensor_scalar_mul(
                        out=ttn[:, s, :], in0=tt[:, s, :],
                        scalar1=rT4[:, s:s + 1])
                d2 = tndram.tile([NJ, D], BF16, tag="d2")
                nc.sync.dma_start(
                    d2.rearrange("(p j) d -> p j d", j=4), ttn)
                tTb = ttp.tile([P, KT, NJ], BF16, tag="tTb")
                nc.sync.dma_start_transpose(tTb, d2)
                nc.vector.tensor_copy(t8[r], tTb)

            def emit_diag():
                """Diag logits over the 1024-row diag slice (2 flat halves);
                products on Pool, reduces on DVE."""
                for h in range(2):
                    ad = dpool.tile([P, 4, D], BF16, tag="ad")
                    nc.sync.dma_start(
                        ad, ad16[ts(h, NJ), :].rearrange(
                            "(p t) d -> p t d", t=4))
                    td = dpool.tile([P, 4, D], BF16, tag="td")
                    nc.sync.dma_start(
                        td, td16[ts(h, NJ), :].rearrange(
                            "(p t) d -> p t d", t=4))
                    for x, y, acc in ((ad, ad, adss), (td, td, tdss),
                                      (ad, td, adot)):
                        scr = dsc.tile([P, 4, D], BF16, tag="dscr4")
                        mul_eng = nc.vector if _nopool() else nc.gpsimd
                        mul_eng.tensor_tensor(out=scr, in0=x, in1=y,
                                              op=OP.mult)
                        with nc.allow_low_precision(reason="bf16 sumsq"):
                            nc.vector.tensor_reduce(
                                out=acc[:, ts(h, 4)], in_=scr, axis=AXX,
                                op=OP.add)
                adlog = tstat.tile([P, DG // P], F32, tag="adlog")
                nc.scalar.activation(adlog, adss, AF.Ln)
                rAd = tstat.tile([P, DG // P], F32, tag="rAd")
                nc.scalar.activation(rAd, adlog, AF.Exp, scale=-0.5,
                                     bias=bias_lnT)
                tdlog = tstat.tile([P, DG // P], F32, tag="tdlog")
                nc.scalar.activation(tdlog, tdss, AF.Ln)
                rTd = tstat.tile([P, DG // P], F32, tag="rTd")
                nc.scalar.activation(rTd, tdlog, AF.Exp, scale=-0.5)
                nc.vector.tensor_mul(diagv, adot, rAd)
                nc.vector.tensor_mul(diagv, diagv, rTd)

            def mm_round(r2):
                """All 32 i-tiles against j-chunks (2*r2, 2*r2+1)."""
                dve_first = pool_first = True
                jslab = ts(r2, 2 * NJ)     # 1024-wide colsum slab
                for ti in range(IT):
                    ps = psum_mm.tile([P, 2, NJ], F32, tag="ps")
                    for k2 in range(K2):
                        for h in range(2):
                            nc.tensor.matmul(
                                ps[:, h, :],
                                aT8[ti // 8][:, 2 * k2:2 * k2 + 2,
                                             ts(ti % 8, P)],
                                t8[2 * r2 + h][:, 2 * k2:2 * k2 + 2, :],
                                start=(k2 == 0), stop=(k2 == K2 - 1),
                                perf_mode=DR)
                    e16 = texp.tile([P, 2, NJ], BF16, tag="e16")
                    nc.scalar.activation(
                        e16, ps, AF.Exp, scale=scaleA[:, ti:ti + 1],
                        accum_out=rs[:, ti, r2:r2 + 1])
                    e16f = e16.rearrange("p a b -> p (a b)")
                    if _cols_engine(ti) == "dve":
                        if dve_first:
                            nc.vector.tensor_copy(cacc_d[:, jslab], e16f)
                            dve_first = False
                        else:
                            nc.vector.tensor_tensor(
                                out=cacc_d[:, jslab],
                                in0=cacc_d[:, jslab], in1=e16f, op=OP.add)
                    else:
                        if pool_first:
                            nc.gpsimd.tensor_copy(cacc_p[:, jslab], e16f)
                            pool_first = False
                        else:
                            nc.gpsimd.tensor_tensor(
                                out=cacc_p[:, jslab],
                                in0=cacc_p[:, jslab], in1=e16f, op=OP.add)
                # partition-reduce the accumulators on PE (per 512 chunk)
                for h in range(2):
                    jc = 2 * r2 + h
                    ps_cs = psum_cs.tile([1, NJ], F32, tag="ps_cs")
                    nc.tensor.matmul(ps_cs, ones16, cacc_d[:, ts(jc, NJ)],
                                     start=True, stop=_nopool(),
                                     skip_group_check=True)
                    if not _nopool():
                        nc.tensor.matmul(ps_cs, ones16,
                                         cacc_p[:, ts(jc, NJ)],
                                         start=False, stop=True,
                                         skip_group_check=True)
                    nc.scalar.copy(colsum_sb[:, ts(jc, NJ)], ps_cs)

            def body():
                t_half(0)
                a_prep(0)
                t_half(1)
                a_prep(1)
                a_prep(2)
                a_prep(3)
                t_half(2)
                t_half(3)
                mm_round(0)
                emit_diag()
                mm_round(1)

                # ---- final reductions ----
                nc.vector.tensor_reduce(out=rsum, in_=rs, axis=AXX,
                                        op=OP.add)
                nc.sync.dma_start(
                    rowsum_out.rearrange("a (t p) -> (a p) t", p=P), rsum)
                nc.sync.dma_start(colsum_out, colsum_sb)
                dsumv = tstat.tile([P, 1], F32, tag="dsumv")
                nc.vector.tensor_reduce(out=dsumv, in_=diagv, axis=AXX,
                                        op=OP.add)
                psf = psum_cs.tile([1, 1], F32, tag="psf")
                nc.tensor.matmul(psf, ones32, dsumv, start=True, stop=True,
                                 skip_group_check=True)
                nc.scalar.copy(scal_sb, psf)
                nc.sync.dma_start(scal_out, scal_sb)

            if repeat > 1:
                with tc.For_i(0, repeat, 1):
                    body()
            else:
                body()


_NC_CACHE = {}


def _build(repeat=1):
    key = (repeat, _nopool())
    if key in _NC_CACHE:
        return _NC_CACHE[key]
    nc = _Bacc("TRN2", target_bir_lowering=False, debug=False,
               num_devices=NCORES)
    a16 = nc.dram_tensor("a16", [AR, D], BF16, kind="ExternalInput").ap()
    t16 = nc.dram_tensor("t16", [TC, D], BF16, kind="ExternalInput").ap()
    ad16 = nc.dram_tensor("ad16", [DG, D], BF16, kind="ExternalInput").ap()
    td16 = nc.dram_tensor("td16", [DG, D], BF16, kind="ExternalInput").ap()
    rowsum_out = nc.dram_tensor("rowsum_out", [1, AR], F32,
                                kind="ExternalOutput").ap()
    colsum_out = nc.dram_tensor("colsum_out", [1, TC], F32,
                                kind="ExternalOutput").ap()
    scal_out = nc.dram_tensor("scal_out", [1, 1], F32,
                              kind="ExternalOutput").ap()
    with tile.TileContext(nc) as tc:
        _emit(tc, a16, t16, ad16, td16, rowsum_out, colsum_out, scal_out,
              repeat=repeat)
    nc.compile()
    _NC_CACHE[key] = nc
    return nc


def make_in_maps(audio_embeds: np.ndarray, text_embeds: np.ndarray):
    a16 = np.asarray(audio_embeds, dtype=np.float32).astype(ml_dtypes.bfloat16)
    t16 = np.asarray(text_embeds, dtype=np.float32).astype(ml_dtypes.bfloat16)
    in_maps = []
    for m in range(NCORES):
        p, q = divmod(m, PC)
        in_maps.append({
            "a16": a16[p * AR:(p + 1) * AR],
            "t16": t16[q * TC:(q + 1) * TC],
            "ad16": a16[m * DG:(m + 1) * DG],
            "td16": t16[m * DG:(m + 1) * DG],
        })
    return in_maps


def combine(results):
    rowsum = np.zeros((B,), np.float64)
    colsum = np.zeros((B,), np.float64)
    diag_sum = 0.0
    for m in range(NCORES):
        p, q = divmod(m, PC)
        rowsum[p * AR:(p + 1) * AR] += \
            results[m]["rowsum_out"].reshape(-1).astype(np.float64)
        colsum[q * TC:(q + 1) * TC] += \
            results[m]["colsum_out"].reshape(-1).astype(np.float64)
        diag_sum += float(results[m]["scal_out"].reshape(-1)[0])
    loss = 0.5 * (np.log(rowsum).mean() + np.log(colsum).mean()) \
        - diag_sum / B
    return np.float32(loss)


def kernel(audio_embeds: np.ndarray, text_embeds: np.ndarray) -> np.ndarray:
    from concourse.bass_utils import run_bass_kernel_spmd

    nc = _build()
    in_maps = make_in_maps(audio_embeds, text_embeds)
    res = run_bass_kernel_spmd(nc, in_maps, list(range(NCORES)))
    return combine(res.results)
